# revision 3
# baseline (speedup 1.0000x reference)
"""Trainium2 Bass kernel for nn_ChoreographModel (conv stack + 2-layer LSTM + FC).

Strategy: pure data-parallel over 8 NeuronCores (batch 4096 -> 512/core).
Per core:
  conv1 (7x3x3->10) as Toeplitz-banded matmuls with dw0/dw1 K-stacked into a
  single K=90 matmul (x staged twice in DRAM, second copy shifted one w),
  maxpool+relu fused on DVE/ACT, conv2 (3x3x10->20) over (h1,c1)=90
  partitions, SBUF shuffle into per-timestep feature-major tiles, then a
  7-step 2-layer LSTM in transposed form zT=[gates, batch].

  LSTM precision: i/f/o gate matmuls run fp8-e4m3 DoubleRow (2x PE rate,
  weights scaled x16, dequant via ACT scale=1/16); the g (candidate) gate
  runs bf16 x bf16 (its error lands additively in the cell state). h is kept
  twice: canonical bf16 tiles + an fp8 pair-packed copy made on the idle
  Pool engine. Cell state fp32.
"""

import sys
from contextlib import ExitStack

if "/opt/trn_rl_repo" not in sys.path:
    sys.path.insert(0, "/opt/trn_rl_repo")

import numpy as np
import ml_dtypes

BF16 = ml_dtypes.bfloat16
F8 = ml_dtypes.float8_e4m3
MM_NAME = "bfloat16"
MM_NP = BF16
N_CORES = 8
WSCALE = 16.0  # fp8 i/f/o weight prescale; dequantized in the ACT

H = 512
NCLS = 256

# conv geometry (hardcoded from the model)
IH, IW, CI = 15, 80, 3
OH1, OW1, CO1 = 9, 78, 10   # after conv1
PW1 = 26                     # after pool1 (78/3)
OH2, OW2, CO2 = 7, 24, 20    # after conv2
PW2 = 8                      # after pool2 (24/3)
T = OH2                      # timesteps
F = PW2 * CO2                # 160 LSTM input features
K1 = IH * CI                 # 45  conv1 contraction rows (x2 stacked = 90)
M1 = OH1 * CO1               # 90  conv1 output rows
K2 = OH1 * CO1               # 90  conv2 contraction rows
M2A, M2B = 4 * CO2, 3 * CO2  # 80/60 conv2 output row groups (oh2 0-3 / 4-6)

NB1 = 6    # conv1 batch chunk (6*78=468 <= 512 psum bank)
NB2 = 21   # conv2 batch chunk (21*24=504 <= 512)

GSLOT = {0: 0, 1: 1, 3: 2}  # gate index (ifgo) -> fp8 pack column slot


def build_nc(B, nsteps=T, reps=1, hw_loop=0, stages=(1, 1, 1), sub=(),
             ps1bufs=6, ps2bufs=3, shuf_engines=3, zbufs=8, gbufs=3,
             dma_spread=0):
    import concourse.bacc as bacc
    import concourse.tile as tile
    from concourse import mybir

    dt = mybir.dt
    AF = mybir.ActivationFunctionType
    DR = mybir.MatmulPerfMode.DoubleRow
    MM = getattr(dt, MM_NAME)
    F8D = dt.float8e4
    TB = T * B

    nc = bacc.Bacc("TRN2", target_bir_lowering=False, debug=False,
                   num_devices=N_CORES)

    x1_d = nc.dram_tensor("x1", [2 * K1, B * IW], MM, kind="ExternalInput")
    w1t2_d = nc.dram_tensor("w1t2", [2 * K1, 2 * M1], MM, kind="ExternalInput")
    cb1_d = nc.dram_tensor("cb1", [M1, 1], dt.float32, kind="ExternalInput")
    w2t_d = nc.dram_tensor("w2t", [K2, 3 * (M2A + M2B)], MM, kind="ExternalInput")
    cb2a_d = nc.dram_tensor("cb2a", [M2A, 1], dt.float32, kind="ExternalInput")
    cb2b_d = nc.dram_tensor("cb2b", [M2B, 1], dt.float32, kind="ExternalInput")
    # fp8 DoubleRow packs for i/f/o gates ([k, 2, 1536] flattened)
    w1p_d = nc.dram_tensor("w1p", [80, 2 * 1536], F8D, kind="ExternalInput")
    u1p_d = [nc.dram_tensor(f"u1p{g}", [128, 2 * 1536], F8D, kind="ExternalInput")
             for g in range(2)]
    w2p_d = [nc.dram_tensor(f"w2p{g}", [128, 2 * 1536], F8D, kind="ExternalInput")
             for g in range(2)]
    u2p_d = [nc.dram_tensor(f"u2p{g}", [128, 2 * 1536], F8D, kind="ExternalInput")
             for g in range(2)]
    # bf16 g-gate weights
    w1ga_d = nc.dram_tensor("w1ga", [128, H], MM, kind="ExternalInput")
    w1gb_d = nc.dram_tensor("w1gb", [F - 128, H], MM, kind="ExternalInput")
    u1g_d = nc.dram_tensor("u1g", [128, 4 * H], MM, kind="ExternalInput")
    w2g_d = nc.dram_tensor("w2g", [128, 4 * H], MM, kind="ExternalInput")
    u2g_d = nc.dram_tensor("u2g", [128, 4 * H], MM, kind="ExternalInput")
    bl_d = nc.dram_tensor("bl", [128, 32], dt.float32, kind="ExternalInput")
    fcw_d = nc.dram_tensor("fcw", [H, NCLS], MM, kind="ExternalInput")
    fcb_d = nc.dram_tensor("fcb", [128, 2], dt.float32, kind="ExternalInput")
    out_d = nc.dram_tensor("out", [NCLS, B], dt.float32, kind="ExternalOutput")

    nch2 = (B + NB2 - 1) // NB2

    with tile.TileContext(nc) as tc:
        with tc.tile_pool(name="consts", bufs=1) as cp, \
             tc.tile_pool(name="seq", bufs=1) as seqp:

            # ---- persistent constants -------------------------------------
            w1t2_s = cp.tile([2 * K1, 2 * M1], MM, tag="w1t2")
            nc.sync.dma_start(w1t2_s[:], w1t2_d[:])
            cb1_s = cp.tile([M1, 1], dt.float32, tag="cb1")
            nc.sync.dma_start(cb1_s[:], cb1_d[:])
            w2t_s = cp.tile([K2, 3 * (M2A + M2B)], MM, tag="w2t")
            nc.sync.dma_start(w2t_s[:], w2t_d[:])
            cb2a_s = cp.tile([M2A, 1], dt.float32, tag="cb2a")
            nc.sync.dma_start(cb2a_s[:], cb2a_d[:])
            cb2b_s = cp.tile([M2B, 1], dt.float32, tag="cb2b")
            nc.sync.dma_start(cb2b_s[:], cb2b_d[:])
            bl_s = cp.tile([128, 32], dt.float32, tag="bl")
            nc.sync.dma_start(bl_s[:], bl_d[:])
            fcb_s = cp.tile([128, 2], dt.float32, tag="fcb")
            nc.sync.dma_start(fcb_s[:], fcb_d[:])

            w1p_s = cp.tile([80, 2 * 1536], F8D, tag="w1p")
            nc.sync.dma_start(w1p_s[:], w1p_d[:])
            u1p_s, w2p_s, u2p_s = [], [], []
            for g in range(2):
                t_ = cp.tile([128, 2 * 1536], F8D, tag=f"u1p{g}")
                nc.gpsimd.dma_start(t_[:], u1p_d[g][:])
                u1p_s.append(t_)
                t_ = cp.tile([128, 2 * 1536], F8D, tag=f"w2p{g}")
                nc.scalar.dma_start(t_[:], w2p_d[g][:])
                w2p_s.append(t_)
                t_ = cp.tile([128, 2 * 1536], F8D, tag=f"u2p{g}")
                nc.gpsimd.dma_start(t_[:], u2p_d[g][:])
                u2p_s.append(t_)

            w1ga_s = cp.tile([128, H], MM, tag="w1ga")
            nc.sync.dma_start(w1ga_s[:], w1ga_d[:])
            w1gb_s = cp.tile([F - 128, H], MM, tag="w1gb")
            nc.sync.dma_start(w1gb_s[:], w1gb_d[:])
            u1g_s = cp.tile([128, 4 * H], MM, tag="u1g")
            nc.scalar.dma_start(u1g_s[:], u1g_d[:])
            w2g_s = cp.tile([128, 4 * H], MM, tag="w2g")
            nc.gpsimd.dma_start(w2g_s[:], w2g_d[:])
            u2g_s = cp.tile([128, 4 * H], MM, tag="u2g")
            nc.scalar.dma_start(u2g_s[:], u2g_d[:])
            fcw_s = []
            for k in range(4):
                fw = cp.tile([128, NCLS], MM, tag=f"fcw_{k}")
                nc.sync.dma_start(fw[:], fcw_d[128 * k:128 * (k + 1), :])
                fcw_s.append(fw)

            import contextlib
            loop_cm = tc.For_i(0, hw_loop, 1) if hw_loop else contextlib.nullcontext()
            with loop_cm:
             for rep in range(reps):
                # ---- LSTM input tiles (filled by conv phases) -----------------
                xhi_s = seqp.tile([128, TB], MM, tag="xhi")
                xlo_s = seqp.tile([F - 128, TB], MM, tag="xlo")
                xpack_s = seqp.tile([80, 2 * TB], F8D, tag="xpack")

                if stages[2] and not stages[1]:
                    nc.gpsimd.memset(xhi_s[:], 0.0)
                    nc.gpsimd.memset(xlo_s[:], 0.0)
                    nc.gpsimd.memset(xpack_s[:], 0.0)
                # ---- conv1 + pool1 --------------------------------------------
                with tc.tile_pool(name="t1p", bufs=1) as t1p, \
                     tc.tile_pool(name="ctmp", bufs=3) as ctp:
                    t1_s = t1p.tile([M1, B * PW1], MM, tag="t1")
                    with tc.tile_pool(name="xin", bufs=1) as xp, \
                         tc.tile_pool(name="ps1", bufs=ps1bufs, space="PSUM") as pp1:
                      if stages[0]:
                          B2 = B // 2 if B % 2 == 0 else B
                          for hb0 in range(0, B, B2):
                              nbh = min(B2, B - hb0)
                              x1h = xp.tile([2 * K1, nbh * IW], MM, tag="x1h",
                                            name="x1h")
                              ndma = 8 if dma_spread else 4
                              dengs = ([nc.sync, nc.gpsimd, nc.scalar]
                                       if dma_spread else [nc.sync])
                              step = ((nbh + ndma - 1) // ndma) * IW
                              for i in range(ndma):
                                  lo = i * step
                                  hi = min((i + 1) * step, nbh * IW)
                                  if lo >= hi:
                                      break
                                  dengs[i % len(dengs)].dma_start(
                                      x1h[:, lo:hi],
                                      x1_d[:, hb0 * IW + lo:hb0 * IW + hi])
                              x1_r = x1h[:].rearrange("p (b w) -> p b w", w=IW)
                              for bl0 in range(0, nbh, NB1):
                                  nb = min(NB1, nbh - bl0)
                                  b0 = hb0 + bl0
                                  if "c1dma" in sub:
                                      continue
                                  ps = pp1.tile([M1, nb * OW1], dt.float32, tag="ps1")
                                  # dw0+dw1 stacked into K=90; dw2 alone K=45
                                  nc.tensor.matmul(
                                      ps[:],
                                      w1t2_s[:, 0:M1],
                                      x1_r[:, bl0:bl0 + nb, 0:OW1],
                                      start=True, stop=False,
                                  )
                                  nc.tensor.matmul(
                                      ps[:],
                                      w1t2_s[0:K1, M1:2 * M1],
                                      x1_r[0:K1, bl0:bl0 + nb, 2:2 + OW1],
                                      start=False, stop=True,
                                  )
                                  if "c1mm" in sub:
                                      continue
                                  mx = ctp.tile([M1, nb * PW1], dt.float32, tag="mx1")
                                  nc.vector.reduce_max(
                                      mx[:],
                                      ps[:].rearrange("p (b q k) -> p b q k",
                                                      q=PW1, k=3),
                                      axis=mybir.AxisListType.X,
                                  )
                                  if "c1red" in sub:
                                      continue
                                  nc.scalar.activation(
                                      t1_s[:, b0 * PW1:(b0 + nb) * PW1], mx[:],
                                      AF.Relu, bias=cb1_s[:],
                                  )

                    # ---- conv2 + pool2 (into (w2-major, b) layout) ------------
                    with tc.tile_pool(name="t2p", bufs=1) as t2p, \
                         tc.tile_pool(name="ps2", bufs=ps2bufs, space="PSUM") as pp2:
                      if stages[1]:
                          t2a_s = t2p.tile([M2A, PW2 * B], MM, tag="t2a")
                          t2b_s = t2p.tile([M2B, PW2 * B], MM, tag="t2b")
                          t1_r = t1_s[:].rearrange("p (b w) -> p b w", w=PW1)
                          t2a_r = t2a_s[:].rearrange("p (w b) -> p b w", b=B)
                          t2b_r = t2b_s[:].rearrange("p (w b) -> p b w", b=B)
                          for c in range(nch2):
                              b0 = c * NB2
                              nb = min(NB2, B - b0)
                              for grp, (off, M, cb_s, t2_r) in enumerate([
                                  (0, M2A, cb2a_s, t2a_r),
                                  (M2A, M2B, cb2b_s, t2b_r),
                              ]):
                                  ps = pp2.tile([M, nb * OW2], dt.float32,
                                                tag=f"ps2_{grp}", name="ps2")
                                  for dw in range(3):
                                      nc.tensor.matmul(
                                          ps[:],
                                          w2t_s[:, dw * 140 + off:dw * 140 + off + M],
                                          t1_r[:, b0:b0 + nb, dw:dw + OW2],
                                          start=(dw == 0), stop=(dw == 2),
                                      )
                                  if "c2mm" in sub:
                                      continue
                                  mx = ctp.tile([M, nb * PW2], dt.float32,
                                                tag=f"mx2_{grp}", name="mx2")
                                  nc.vector.reduce_max(
                                      mx[:],
                                      ps[:].rearrange("p (b q k) -> p b q k",
                                                      q=PW2, k=3),
                                      axis=mybir.AxisListType.X,
                                  )
                                  if "c2red" in sub:
                                      continue
                                  nc.scalar.activation(
                                      t2_r[:, b0:b0 + nb, :],
                                      mx[:].rearrange("p (b w) -> p b w", w=PW2),
                                      AF.Relu, bias=cb_s[:],
                                  )

                          # ---- shuffle into per-timestep feature-major tiles ----
                          for t in range(T if "noshuf" not in sub else 0):
                              src = t2a_s if t < 4 else t2b_s
                              rs = (t % 4) * CO2 if t < 4 else (t - 4) * CO2
                              for w2 in range(PW2):
                                  r0 = w2 * CO2
                                  spans = []
                                  if r0 + CO2 <= 128:
                                      spans.append((xhi_s, r0, 0, CO2))
                                  elif r0 >= 128:
                                      spans.append((xlo_s, r0 - 128, 0, CO2))
                                  else:
                                      spans.append((xhi_s, r0, 0, 128 - r0))
                                      spans.append((xlo_s, 0, 128 - r0, CO2 - (128 - r0)))
                                  engs = [nc.sync, nc.gpsimd,
                                          nc.scalar][:shuf_engines]
                                  for dst, d0, s0, n in spans:
                                      eng = engs[(t * PW2 + w2) % len(engs)]
                                      eng.dma_start(
                                          dst[d0:d0 + n, t * B:(t + 1) * B],
                                          src[rs + s0:rs + s0 + n,
                                              w2 * B:(w2 + 1) * B],
                                      )

                          # ---- fp8 x-pack: convert + repack for DoubleRow ----
                          if "noxpack" not in sub:
                              xhi8 = ctp.tile([128, TB], F8D, tag="xhi8",
                                              name="xhi8")
                              xlo8 = ctp.tile([F - 128, TB], F8D, tag="xlo8",
                                              name="xlo8")
                              nc.gpsimd.tensor_copy(xhi8[:], xhi_s[:])
                              nc.gpsimd.tensor_copy(xlo8[:], xlo_s[:])
                              nc.sync.dma_start(xpack_s[:, 0:TB], xhi8[0:80, :])
                              nc.scalar.dma_start(xpack_s[0:48, TB:2 * TB],
                                                  xhi8[80:128, :])
                              nc.gpsimd.dma_start(xpack_s[48:80, TB:2 * TB],
                                                  xlo8[0:F - 128, :])

                if stages[2]:
                  # ---- LSTM ----------------------------------------------------
                  ls = ExitStack()
                  stp = ls.enter_context(tc.tile_pool(name="state", bufs=1))
                  gp = ls.enter_context(tc.tile_pool(name="gates", bufs=gbufs))
                  h1_t = [[stp.tile([128, B], MM, tag=f"h1_{p}_{j}", name=f"h1_{p}_{j}")
                           for j in range(4)] for p in range(2)]
                  h2_t = [[stp.tile([128, B], MM, tag=f"h2_{p}_{j}", name=f"h2_{p}_{j}")
                           for j in range(4)] for p in range(2)]
                  # fp8 pair-packed h copies (dims (256g+128i+k) at [k, i*B+n])
                  h1p_t = [[stp.tile([128, 2 * B], F8D, tag=f"h1p_{p}_{g}",
                                     name=f"h1p_{p}_{g}") for g in range(2)]
                           for p in range(2)]
                  h2p_t = [[stp.tile([128, 2 * B], F8D, tag=f"h2p_{p}_{g}",
                                     name=f"h2p_{p}_{g}") for g in range(2)]
                           for p in range(2)]
                  c1_t = [stp.tile([128, B], dt.float32, tag=f"c1_{j}",
                                   name=f"c1_{j}") for j in range(4)]
                  c2_t = [stp.tile([128, B], dt.float32, tag=f"c2_{j}",
                                   name=f"c2_{j}") for j in range(4)]

                  xp_r = xpack_s[:].rearrange("p (i f) -> p i f", i=2)
                  w1p_r = w1p_s[:].rearrange("p (i f) -> p i f", i=2)
                  u1p_r = [u1p_s[g][:].rearrange("p (i f) -> p i f", i=2)
                           for g in range(2)]
                  w2p_r = [w2p_s[g][:].rearrange("p (i f) -> p i f", i=2)
                           for g in range(2)]
                  u2p_r = [u2p_s[g][:].rearrange("p (i f) -> p i f", i=2)
                           for g in range(2)]

                  with tc.tile_pool(name="zps", bufs=zbufs, space="PSUM") as zpp:
                      for t in range(nsteps):
                          h1_prev, h1_new = h1_t[t % 2], h1_t[(t + 1) % 2]
                          h2_prev, h2_new = h2_t[t % 2], h2_t[(t + 1) % 2]
                          h1p_prev, h1p_new = h1p_t[t % 2], h1p_t[(t + 1) % 2]
                          h2p_prev, h2p_new = h2p_t[t % 2], h2p_t[(t + 1) % 2]
                          for layer in range(2):
                              h_prev = h1_prev if layer == 0 else h2_prev
                              h_new = h1_new if layer == 0 else h2_new
                              hp_prev = h1p_prev if layer == 0 else h2p_prev
                              hp_new = h1p_new if layer == 0 else h2p_new
                              c_t = c1_t if layer == 0 else c2_t
                              bcol = 0 if layer == 0 else 16
                              s_g4 = {}
                              for j in range(4):
                                  for gi, gname in enumerate("ifgo"):
                                      m = gi * 4 + j
                                      ps = zpp.tile([128, B], dt.float32, tag="z")
                                      if gname == "g":
                                          col = j * 128
                                          if layer == 0:
                                              nc.tensor.matmul(
                                                  ps[:], w1ga_s[:, col:col + 128],
                                                  xhi_s[:, t * B:(t + 1) * B],
                                                  start=True, stop=False)
                                              nc.tensor.matmul(
                                                  ps[:], w1gb_s[:, col:col + 128],
                                                  xlo_s[:, t * B:(t + 1) * B],
                                                  start=False, stop=(t == 0))
                                              if t > 0:
                                                  for k in range(4):
                                                      nc.tensor.matmul(
                                                          ps[:],
                                                          u1g_s[:, k * H + col:
                                                                k * H + col + 128],
                                                          h_prev[k][:],
                                                          start=False, stop=(k == 3))
                                          else:
                                              if t > 0:
                                                  for k in range(4):
                                                      nc.tensor.matmul(
                                                          ps[:],
                                                          u2g_s[:, k * H + col:
                                                                k * H + col + 128],
                                                          h_prev[k][:],
                                                          start=(k == 0), stop=False)
                                              for k in range(4):
                                                  nc.tensor.matmul(
                                                      ps[:],
                                                      w2g_s[:, k * H + col:
                                                            k * H + col + 128],
                                                      h1_new[k][:],
                                                      start=(t == 0 and k == 0),
                                                      stop=(k == 3))
                                      else:
                                          # fp8 DoubleRow path: out [128, 256]
                                          # per n-half, K pairs packed in dim1
                                          c0 = GSLOT[gi] * 512 + j * 128
                                          for nh in range(2):
                                              qd = ps[:, nh * 256:nh * 256 + 256]
                                              n0 = nh * 256
                                              if layer == 0:
                                                  nc.tensor.matmul(
                                                      qd,
                                                      w1p_r[:, :, c0:c0 + 128],
                                                      xp_r[:, :, t * B + n0:
                                                           t * B + n0 + 256],
                                                      start=True,
                                                      stop=(t == 0),
                                                      perf_mode=DR)
                                                  if t > 0:
                                                      for g in range(2):
                                                          nc.tensor.matmul(
                                                              qd,
                                                              u1p_r[g][:, :,
                                                                    c0:c0 + 128],
                                                              hp_prev[g][:]
                                                              .rearrange(
                                                                  "p (i b) -> p i b",
                                                                  i=2)
                                                              [:, :, n0:n0 + 256],
                                                              start=False,
                                                              stop=(g == 1),
                                                              perf_mode=DR)
                                              else:
                                                  if t > 0:
                                                      for g in range(2):
                                                          nc.tensor.matmul(
                                                              qd,
                                                              u2p_r[g][:, :,
                                                                    c0:c0 + 128],
                                                              hp_prev[g][:]
                                                              .rearrange(
                                                                  "p (i b) -> p i b",
                                                                  i=2)
                                                              [:, :, n0:n0 + 256],
                                                              start=(g == 0),
                                                              stop=False,
                                                              perf_mode=DR)
                                                  for g in range(2):
                                                      nc.tensor.matmul(
                                                          qd,
                                                          w2p_r[g][:, :,
                                                                c0:c0 + 128],
                                                          h1p_new[g][:]
                                                          .rearrange(
                                                              "p (i b) -> p i b",
                                                              i=2)
                                                          [:, :, n0:n0 + 256],
                                                          start=(t == 0 and
                                                                 g == 0),
                                                          stop=(g == 1),
                                                          perf_mode=DR)
                                      func = AF.Tanh if gname == "g" else AF.Sigmoid
                                      s = gp.tile([128, B], dt.float32,
                                                  tag=f"s_{gname}")
                                      nc.scalar.activation(
                                          s[:], ps[:], func,
                                          bias=bl_s[:, bcol + m:bcol + m + 1],
                                          scale=(1.0 if gname == "g"
                                                 else 1.0 / WSCALE))
                                      s_g4[gname] = s
                                  # cell update for H-tile j
                                  si, sf, sg, so = (s_g4[g] for g in "ifgo")
                                  if t == 0:
                                      nc.vector.tensor_mul(c_t[j][:], si[:], sg[:])
                                  else:
                                      tig = gp.tile([128, B], dt.float32, tag="tig")
                                      nc.vector.tensor_mul(tig[:], si[:], sg[:])
                                      nc.vector.tensor_mul(c_t[j][:], c_t[j][:], sf[:])
                                      nc.vector.tensor_add(c_t[j][:], c_t[j][:], tig[:])
                                  tc_ = gp.tile([128, B], dt.float32, tag="tc")
                                  nc.scalar.activation(tc_[:], c_t[j][:], AF.Tanh)
                                  nc.vector.tensor_mul(h_new[j][:], so[:], tc_[:])
                                  # fp8 pair-packed copy on the Pool engine
                                  if t < nsteps - 1 or layer == 0:
                                      nc.gpsimd.tensor_copy(
                                          hp_new[j // 2][:, (j % 2) * B:
                                                         (j % 2 + 1) * B],
                                          h_new[j][:])

                      # ---- FC + relu ------------------------------------------
                      for mo in range(2):
                          ps = zpp.tile([128, B], dt.float32, tag="z")
                          for k in range(4):
                              nc.tensor.matmul(
                                  ps[:], fcw_s[k][:, mo * 128:(mo + 1) * 128],
                                  h2_t[nsteps % 2][k][:], start=(k == 0), stop=(k == 3))
                          o_s = gp.tile([128, B], dt.float32, tag="o")
                          nc.scalar.activation(o_s[:], ps[:], AF.Relu,
                                               bias=fcb_s[:, mo:mo + 1])
                          nc.sync.dma_start(out_d[mo * 128:(mo + 1) * 128, :], o_s[:])
                  ls.close()

    nc.finalize()
    return nc


def _pack_dr(Wsel, kparts):
    """[2*kparts, 1536] fp32 -> DoubleRow pack [kparts, 2*1536] fp8."""
    blk = Wsel.reshape(2, kparts, Wsel.shape[1]).transpose(1, 0, 2)
    return np.ascontiguousarray(
        np.clip(blk.reshape(kparts, -1) * WSCALE, -240, 240)).astype(F8)


def _ifo_cols(W):
    """columns for gates i, f, o in pack slot order."""
    return np.concatenate([W[:, 0:H], W[:, H:2 * H], W[:, 3 * H:4 * H]], axis=1)


def prep_consts(conv1_w, conv1_b, conv2_w, conv2_b, W1, U1, b1, W2, U2, b2,
                fc_w, fc_b):
    w1t = np.zeros((K1, 3 * M1), np.float32)
    for dw in range(3):
        for oh in range(OH1):
            for dh in range(7):
                w1t[(oh + dh) * CI:(oh + dh + 1) * CI,
                    dw * M1 + oh * CO1:dw * M1 + (oh + 1) * CO1] = conv1_w[dh, dw]
    # K-stacked conv1 weights: [90, 180]; block A = dw0 (rows 0:45) + dw1
    # (rows 45:90); block B = dw2 in rows 0:45.
    w1t2 = np.zeros((2 * K1, 2 * M1), np.float32)
    w1t2[0:K1, 0:M1] = w1t[:, 0:M1]
    w1t2[K1:2 * K1, 0:M1] = w1t[:, M1:2 * M1]
    w1t2[0:K1, M1:2 * M1] = w1t[:, 2 * M1:3 * M1]

    w2t = np.zeros((K2, 3 * (M2A + M2B)), np.float32)
    for dw in range(3):
        for oh in range(OH2):
            for dh in range(3):
                w2t[(oh + dh) * CO1:(oh + dh + 1) * CO1,
                    dw * 140 + oh * CO2:dw * 140 + (oh + 1) * CO2] = conv2_w[dh, dw]

    W1f, U1f = np.asarray(W1, np.float32), np.asarray(U1, np.float32)
    W2f, U2f = np.asarray(W2, np.float32), np.asarray(U2, np.float32)

    out = {
        "w1t2": w1t2.astype(MM_NP),
        "cb1": np.tile(conv1_b, OH1)[:, None].astype(np.float32),
        "w2t": w2t.astype(MM_NP),
        "cb2a": np.tile(conv2_b, 4)[:, None].astype(np.float32),
        "cb2b": np.tile(conv2_b, 3)[:, None].astype(np.float32),
        "w1p": _pack_dr(_ifo_cols(W1f), 80),
        "w1ga": W1f[0:128, 2 * H:3 * H].astype(MM_NP),
        "w1gb": W1f[128:F, 2 * H:3 * H].astype(MM_NP),
        "bl": np.concatenate([np.asarray(b1).reshape(16, 128).T,
                              np.asarray(b2).reshape(16, 128).T],
                             axis=1).astype(np.float32),
        "fcw": np.asarray(fc_w, np.float32).astype(MM_NP),
        "fcb": np.asarray(fc_b).reshape(2, 128).T.astype(np.float32),
    }
    for nm, Mf in (("u1p", U1f), ("w2p", W2f), ("u2p", U2f)):
        sel = _ifo_cols(Mf)
        for g in range(2):
            out[f"{nm}{g}"] = _pack_dr(sel[256 * g:256 * (g + 1)], 128)
    # bf16 g-gate recurrent weights: [128, 4*H] with k-tile-major columns
    for nm, Mf in (("u1g", U1f), ("w2g", W2f), ("u2g", U2f)):
        gcols = Mf[:, 2 * H:3 * H]           # [512, 512]
        kt = gcols.reshape(4, 128, H).transpose(1, 0, 2).reshape(128, 4 * H)
        out[nm] = np.ascontiguousarray(kt).astype(MM_NP)
    return out


def prep_x(x_shard):
    B = x_shard.shape[0]
    xr = np.asarray(x_shard, np.float32).transpose(1, 3, 0, 2).reshape(K1, B, IW)
    x2 = np.zeros((2 * K1, B, IW), np.float32)
    x2[0:K1] = xr
    x2[K1:, :, 0:IW - 1] = xr[:, :, 1:]      # w+1 shifted copy for dw1
    return np.ascontiguousarray(x2.reshape(2 * K1, B * IW)).astype(MM_NP)


_NC_CACHE = {}


def _get_nc(B):
    if B not in _NC_CACHE:
        _NC_CACHE[B] = build_nc(B)
    return _NC_CACHE[B]


def kernel(**inputs):
    from concourse.bass_utils import run_bass_kernel_spmd

    x = np.asarray(inputs["x"])
    Bfull = x.shape[0]
    B = Bfull // N_CORES
    nc = _get_nc(B)
    consts = prep_consts(**{k: np.asarray(v) for k, v in inputs.items()
                            if k != "x"})
    in_maps = []
    for c in range(N_CORES):
        m = dict(consts)
        m["x1"] = prep_x(x[c * B:(c + 1) * B])
        in_maps.append(m)
    res = run_bass_kernel_spmd(nc, in_maps, list(range(N_CORES)))
    out = np.concatenate(
        [res.results[c]["out"].T for c in range(N_CORES)], axis=0)
    return np.ascontiguousarray(out.astype(np.float32))


# revision 17
# speedup vs baseline: 1.4568x; 1.4568x over previous
"""Trainium2 Bass kernel for nn_ChoreographModel (conv stack + 2-layer LSTM + FC).

Strategy: pure data-parallel over 8 NeuronCores (batch 4096 -> 512/core).
Per core:
  conv1 (7x3x3->10) as Toeplitz-banded matmuls with dw0/dw1 K-stacked into a
  single K=90 matmul (x staged twice in DRAM, second copy shifted one w),
  maxpool+relu fused on DVE/ACT, conv2 (3x3x10->20) over (h1,c1)=90
  partitions, SBUF shuffle into per-timestep feature-major tiles, then a
  7-step 2-layer LSTM in transposed form zT=[gates, batch].

  LSTM precision: i/f/o gate matmuls run fp8-e4m3 DoubleRow (2x PE rate,
  weights scaled x16, dequant via ACT scale=1/16); the g (candidate) gate
  runs bf16 x bf16 (its error lands additively in the cell state). h is kept
  twice: canonical bf16 tiles + an fp8 pair-packed copy made on the idle
  Pool engine. Cell state fp32.
"""

import sys
from contextlib import ExitStack

if "/opt/trn_rl_repo" not in sys.path:
    sys.path.insert(0, "/opt/trn_rl_repo")

import numpy as np
import ml_dtypes

BF16 = ml_dtypes.bfloat16
F8 = ml_dtypes.float8_e4m3
MM_NAME = "bfloat16"
MM_NP = BF16
N_CORES = 8
WSCALE = 16.0  # fp8 i/f/o weight prescale; dequantized in the ACT

H = 512
NCLS = 256

# conv geometry (hardcoded from the model)
IH, IW, CI = 15, 80, 3
OH1, OW1, CO1 = 9, 78, 10   # after conv1
PW1 = 26                     # after pool1 (78/3)
OH2, OW2, CO2 = 7, 24, 20    # after conv2
PW2 = 8                      # after pool2 (24/3)
T = OH2                      # timesteps
F = PW2 * CO2                # 160 LSTM input features
K1 = IH * CI                 # 45  conv1 contraction rows (x2 stacked = 90)
M1 = OH1 * CO1               # 90  conv1 output rows
K2 = OH1 * CO1               # 90  conv2 contraction rows
M2A, M2B = 4 * CO2, 3 * CO2  # 80/60 conv2 output row groups (oh2 0-3 / 4-6)

NB1 = 6    # conv1 batch chunk (6*78=468 <= 512 psum bank)
NB2 = 21   # conv2 batch chunk (21*24=504 <= 512)

GSLOT = {0: 0, 1: 1, 3: 2}  # gate index (ifgo) -> fp8 pack column slot


def build_nc(B, nsteps=T, reps=1, hw_loop=0, stages=(1, 1, 1), sub=(),
             ps1bufs=6, ps2bufs=3, shuf_engines=3, zbufs=8, gbufs=3,
             dma_spread=0):
    import concourse.bacc as bacc
    import concourse.tile as tile
    from concourse import mybir

    dt = mybir.dt
    AF = mybir.ActivationFunctionType
    DR = mybir.MatmulPerfMode.DoubleRow
    MM = getattr(dt, MM_NAME)
    F8D = dt.float8e4
    TB = T * B

    nc = bacc.Bacc("TRN2", target_bir_lowering=False, debug=False,
                   num_devices=N_CORES)

    x1_d = nc.dram_tensor("x1", [2 * K1, B * IW], MM, kind="ExternalInput")
    w1t2_d = nc.dram_tensor("w1t2", [2 * K1, 2 * M1], MM, kind="ExternalInput")
    cb1_d = nc.dram_tensor("cb1", [M1, 1], dt.float32, kind="ExternalInput")
    w2t_d = nc.dram_tensor("w2t", [K2, 3 * (M2A + M2B)], MM, kind="ExternalInput")
    cb2a_d = nc.dram_tensor("cb2a", [M2A, 1], dt.float32, kind="ExternalInput")
    cb2b_d = nc.dram_tensor("cb2b", [M2B, 1], dt.float32, kind="ExternalInput")
    # fp8 DoubleRow packs for i/f/o gates ([k, 2, 1536] flattened)
    w1p_d = nc.dram_tensor("w1p", [80, 2 * 1536], F8D, kind="ExternalInput")
    u1p_d = [nc.dram_tensor(f"u1p{g}", [128, 2 * 1536], F8D, kind="ExternalInput")
             for g in range(2)]
    w2p_d = [nc.dram_tensor(f"w2p{g}", [128, 2 * 1536], F8D, kind="ExternalInput")
             for g in range(2)]
    u2p_d = [nc.dram_tensor(f"u2p{g}", [128, 2 * 1536], F8D, kind="ExternalInput")
             for g in range(2)]
    # bf16 g-gate weights
    w1ga_d = nc.dram_tensor("w1ga", [128, H], MM, kind="ExternalInput")
    w1gb_d = nc.dram_tensor("w1gb", [F - 128, H], MM, kind="ExternalInput")
    u1g_d = nc.dram_tensor("u1g", [128, 4 * H], MM, kind="ExternalInput")
    w2g_d = nc.dram_tensor("w2g", [128, 4 * H], MM, kind="ExternalInput")
    u2g_d = nc.dram_tensor("u2g", [128, 4 * H], MM, kind="ExternalInput")
    bl_d = nc.dram_tensor("bl", [128, 32], dt.float32, kind="ExternalInput")
    fcw_d = nc.dram_tensor("fcw", [H, NCLS], MM, kind="ExternalInput")
    fcb_d = nc.dram_tensor("fcb", [128, 2], dt.float32, kind="ExternalInput")
    out_d = nc.dram_tensor("out", [NCLS, B], dt.float32, kind="ExternalOutput")

    nch2 = (B + NB2 - 1) // NB2

    with tile.TileContext(nc) as tc:
        with tc.tile_pool(name="consts", bufs=1) as cp, \
             tc.tile_pool(name="seq", bufs=1) as seqp:

            # ---- persistent constants -------------------------------------
            w1t2_s = cp.tile([2 * K1, 2 * M1], MM, tag="w1t2")
            nc.sync.dma_start(w1t2_s[:], w1t2_d[:])
            cb1_s = cp.tile([M1, 1], dt.float32, tag="cb1")
            nc.sync.dma_start(cb1_s[:], cb1_d[:])
            w2t_s = cp.tile([K2, 3 * (M2A + M2B)], MM, tag="w2t")
            nc.sync.dma_start(w2t_s[:], w2t_d[:])
            cb2a_s = cp.tile([M2A, 1], dt.float32, tag="cb2a")
            nc.sync.dma_start(cb2a_s[:], cb2a_d[:])
            cb2b_s = cp.tile([M2B, 1], dt.float32, tag="cb2b")
            nc.sync.dma_start(cb2b_s[:], cb2b_d[:])
            bl_s = cp.tile([128, 32], dt.float32, tag="bl")
            nc.sync.dma_start(bl_s[:], bl_d[:])
            fcb_s = cp.tile([128, 2], dt.float32, tag="fcb")
            nc.sync.dma_start(fcb_s[:], fcb_d[:])

            w1p_s = cp.tile([80, 2 * 1536], F8D, tag="w1p")
            nc.sync.dma_start(w1p_s[:], w1p_d[:])
            u1p_s, w2p_s, u2p_s = [], [], []
            for g in range(2):
                t_ = cp.tile([128, 2 * 1536], F8D, tag=f"u1p{g}")
                nc.gpsimd.dma_start(t_[:], u1p_d[g][:])
                u1p_s.append(t_)
                t_ = cp.tile([128, 2 * 1536], F8D, tag=f"w2p{g}")
                nc.scalar.dma_start(t_[:], w2p_d[g][:])
                w2p_s.append(t_)
                t_ = cp.tile([128, 2 * 1536], F8D, tag=f"u2p{g}")
                nc.gpsimd.dma_start(t_[:], u2p_d[g][:])
                u2p_s.append(t_)

            w1ga_s = cp.tile([128, H], MM, tag="w1ga")
            nc.sync.dma_start(w1ga_s[:], w1ga_d[:])
            w1gb_s = cp.tile([F - 128, H], MM, tag="w1gb")
            nc.sync.dma_start(w1gb_s[:], w1gb_d[:])
            u1g_s = cp.tile([128, 4 * H], MM, tag="u1g")
            nc.scalar.dma_start(u1g_s[:], u1g_d[:])
            w2g_s = cp.tile([128, 4 * H], MM, tag="w2g")
            nc.gpsimd.dma_start(w2g_s[:], w2g_d[:])
            u2g_s = cp.tile([128, 4 * H], MM, tag="u2g")
            nc.scalar.dma_start(u2g_s[:], u2g_d[:])
            fcw_s = []
            for k in range(4):
                fw = cp.tile([128, NCLS], MM, tag=f"fcw_{k}")
                nc.sync.dma_start(fw[:], fcw_d[128 * k:128 * (k + 1), :])
                fcw_s.append(fw)

            import contextlib
            loop_cm = tc.For_i(0, hw_loop, 1) if hw_loop else contextlib.nullcontext()
            with loop_cm:
             for rep in range(reps):
                # ---- LSTM input tiles (filled by conv phases) -----------------
                xhi_s = seqp.tile([128, TB], MM, tag="xhi")
                xlo_s = seqp.tile([F - 128, TB], MM, tag="xlo")
                xpack_s = seqp.tile([80, 2 * TB], F8D, tag="xpack")

                if stages[2] and not stages[1]:
                    nc.gpsimd.memset(xhi_s[:], 0.0)
                    nc.gpsimd.memset(xlo_s[:], 0.0)
                    nc.gpsimd.memset(xpack_s[:], 0.0)
                # ---- conv1 + pool1 --------------------------------------------
                with tc.tile_pool(name="t1p", bufs=1) as t1p, \
                     tc.tile_pool(name="ctmp", bufs=3) as ctp:
                    t1_s = t1p.tile([M1, B * PW1], MM, tag="t1")
                    with tc.tile_pool(name="xin", bufs=1) as xp, \
                         tc.tile_pool(name="ps1", bufs=ps1bufs, space="PSUM") as pp1:
                      if stages[0]:
                          B2 = B // 2 if B % 2 == 0 else B
                          for hb0 in range(0, B, B2):
                              nbh = min(B2, B - hb0)
                              x1h = xp.tile([2 * K1, nbh * IW], MM, tag="x1h",
                                            name="x1h")
                              ndma = 8 if dma_spread else 4
                              dengs = ([nc.sync, nc.gpsimd, nc.scalar]
                                       if dma_spread else [nc.sync])
                              step = ((nbh + ndma - 1) // ndma) * IW
                              for i in range(ndma):
                                  lo = i * step
                                  hi = min((i + 1) * step, nbh * IW)
                                  if lo >= hi:
                                      break
                                  dengs[i % len(dengs)].dma_start(
                                      x1h[:, lo:hi],
                                      x1_d[:, hb0 * IW + lo:hb0 * IW + hi])
                              x1_r = x1h[:].rearrange("p (b w) -> p b w", w=IW)
                              for bl0 in range(0, nbh, NB1):
                                  nb = min(NB1, nbh - bl0)
                                  b0 = hb0 + bl0
                                  if "c1dma" in sub:
                                      continue
                                  ps = pp1.tile([M1, nb * OW1], dt.float32, tag="ps1")
                                  # dw0+dw1 stacked into K=90 (+bias row);
                                  # dw2 alone K=45
                                  nc.tensor.matmul(
                                      ps[:],
                                      w1t2_s[:, 0:M1],
                                      x1_r[:, bl0:bl0 + nb, 0:OW1],
                                      start=True, stop=False,
                                  )
                                  nc.tensor.matmul(
                                      ps[:],
                                      w1t2_s[0:K1, M1:2 * M1],
                                      x1_r[0:K1, bl0:bl0 + nb, 2:2 + OW1],
                                      start=False, stop=True,
                                  )
                                  if "c1mm" in sub:
                                      continue
                                  mx = ctp.tile([M1, nb * PW1], dt.float32, tag="mx1")
                                  nc.vector.reduce_max(
                                      mx[:],
                                      ps[:].rearrange("p (b q k) -> p b q k",
                                                      q=PW1, k=3),
                                      axis=mybir.AxisListType.X,
                                  )
                                  if "c1red" in sub:
                                      continue
                                  nc.scalar.activation(
                                      t1_s[:, b0 * PW1:(b0 + nb) * PW1], mx[:],
                                      AF.Relu, bias=cb1_s[:],
                                  )

                    # ---- conv2 + pool2 (into (w2-major, b) layout) ------------
                    with tc.tile_pool(name="t2p", bufs=1) as t2p, \
                         tc.tile_pool(name="ps2", bufs=ps2bufs, space="PSUM") as pp2:
                      if stages[1]:
                          t2a_s = t2p.tile([M2A, PW2 * B], MM, tag="t2a")
                          t2b_s = t2p.tile([M2B, PW2 * B], MM, tag="t2b")
                          t1_r = t1_s[:].rearrange("p (b w) -> p b w", w=PW1)
                          t2a_r = t2a_s[:].rearrange("p (w b) -> p b w", b=B)
                          t2b_r = t2b_s[:].rearrange("p (w b) -> p b w", b=B)
                          for c in range(nch2):
                              b0 = c * NB2
                              nb = min(NB2, B - b0)
                              for grp, (off, M, cb_s, t2_r) in enumerate([
                                  (0, M2A, cb2a_s, t2a_r),
                                  (M2A, M2B, cb2b_s, t2b_r),
                              ]):
                                  ps = pp2.tile([M, nb * OW2], dt.float32,
                                                tag=f"ps2_{grp}", name="ps2")
                                  for dw in range(3):
                                      nc.tensor.matmul(
                                          ps[:],
                                          w2t_s[:, dw * 140 + off:dw * 140 + off + M],
                                          t1_r[:, b0:b0 + nb, dw:dw + OW2],
                                          start=(dw == 0), stop=(dw == 2),
                                      )
                                  if "c2mm" in sub:
                                      continue
                                  mx = ctp.tile([M, nb * PW2], dt.float32,
                                                tag=f"mx2_{grp}", name="mx2")
                                  nc.vector.reduce_max(
                                      mx[:],
                                      ps[:].rearrange("p (b q k) -> p b q k",
                                                      q=PW2, k=3),
                                      axis=mybir.AxisListType.X,
                                  )
                                  if "c2red" in sub:
                                      continue
                                  nc.scalar.activation(
                                      t2_r[:, b0:b0 + nb, :],
                                      mx[:].rearrange("p (b w) -> p b w", w=PW2),
                                      AF.Relu, bias=cb_s[:],
                                  )

                          # ---- shuffle into per-timestep feature-major tiles ----
                          for t in range(T if "noshuf" not in sub else 0):
                              src = t2a_s if t < 4 else t2b_s
                              rs = (t % 4) * CO2 if t < 4 else (t - 4) * CO2
                              for w2 in range(PW2):
                                  r0 = w2 * CO2
                                  spans = []
                                  if r0 + CO2 <= 128:
                                      spans.append((xhi_s, r0, 0, CO2))
                                  elif r0 >= 128:
                                      spans.append((xlo_s, r0 - 128, 0, CO2))
                                  else:
                                      spans.append((xhi_s, r0, 0, 128 - r0))
                                      spans.append((xlo_s, 0, 128 - r0, CO2 - (128 - r0)))
                                  engs = [nc.sync, nc.gpsimd,
                                          nc.scalar][:shuf_engines]
                                  for dst, d0, s0, n in spans:
                                      eng = engs[(t * PW2 + w2) % len(engs)]
                                      eng.dma_start(
                                          dst[d0:d0 + n, t * B:(t + 1) * B],
                                          src[rs + s0:rs + s0 + n,
                                              w2 * B:(w2 + 1) * B],
                                      )

                          # ---- fp8 x-pack: convert + repack for DoubleRow ----
                          if "noxpack" not in sub:
                              xhi8 = ctp.tile([128, TB], F8D, tag="xhi8",
                                              name="xhi8")
                              xlo8 = ctp.tile([F - 128, TB], F8D, tag="xlo8",
                                              name="xlo8")
                              nc.gpsimd.tensor_copy(xhi8[:], xhi_s[:])
                              nc.gpsimd.tensor_copy(xlo8[:], xlo_s[:])
                              nc.sync.dma_start(xpack_s[:, 0:TB], xhi8[0:80, :])
                              nc.scalar.dma_start(xpack_s[0:48, TB:2 * TB],
                                                  xhi8[80:128, :])
                              nc.gpsimd.dma_start(xpack_s[48:80, TB:2 * TB],
                                                  xlo8[0:F - 128, :])

                if stages[2]:
                  # ---- LSTM ----------------------------------------------------
                  ls = ExitStack()
                  stp = ls.enter_context(tc.tile_pool(name="state", bufs=1))
                  gp = ls.enter_context(tc.tile_pool(name="gates", bufs=gbufs))
                  h1_t = [[stp.tile([128, B], MM, tag=f"h1_{p}_{j}", name=f"h1_{p}_{j}")
                           for j in range(4)] for p in range(2)]
                  h2_t = [[stp.tile([128, B], MM, tag=f"h2_{p}_{j}", name=f"h2_{p}_{j}")
                           for j in range(4)] for p in range(2)]
                  # fp8 pair-packed h copies (dims (256g+128i+k) at [k, i*B+n])
                  h1p_t = [[stp.tile([128, 2 * B], F8D, tag=f"h1p_{p}_{g}",
                                     name=f"h1p_{p}_{g}") for g in range(2)]
                           for p in range(2)]
                  h2p_t = [[stp.tile([128, 2 * B], F8D, tag=f"h2p_{p}_{g}",
                                     name=f"h2p_{p}_{g}") for g in range(2)]
                           for p in range(2)]
                  c1_t = [stp.tile([128, B], dt.float32, tag=f"c1_{j}",
                                   name=f"c1_{j}") for j in range(4)]
                  c2_t = [stp.tile([128, B], dt.float32, tag=f"c2_{j}",
                                   name=f"c2_{j}") for j in range(4)]

                  xp_r = xpack_s[:].rearrange("p (i f) -> p i f", i=2)
                  w1p_r = w1p_s[:].rearrange("p (i f) -> p i f", i=2)
                  u1p_r = [u1p_s[g][:].rearrange("p (i f) -> p i f", i=2)
                           for g in range(2)]
                  w2p_r = [w2p_s[g][:].rearrange("p (i f) -> p i f", i=2)
                           for g in range(2)]
                  u2p_r = [u2p_s[g][:].rearrange("p (i f) -> p i f", i=2)
                           for g in range(2)]

                  with tc.tile_pool(name="zps", bufs=zbufs, space="PSUM") as zpp:
                      for t in range(nsteps):
                          h1_prev, h1_new = h1_t[t % 2], h1_t[(t + 1) % 2]
                          h2_prev, h2_new = h2_t[t % 2], h2_t[(t + 1) % 2]
                          h1p_prev, h1p_new = h1p_t[t % 2], h1p_t[(t + 1) % 2]
                          h2p_prev, h2p_new = h2p_t[t % 2], h2p_t[(t + 1) % 2]
                          for layer in range(2):
                              h_prev = h1_prev if layer == 0 else h2_prev
                              h_new = h1_new if layer == 0 else h2_new
                              hp_prev = h1p_prev if layer == 0 else h2p_prev
                              hp_new = h1p_new if layer == 0 else h2p_new
                              c_t = c1_t if layer == 0 else c2_t
                              bcol = 0 if layer == 0 else 16
                              s_g4 = {}
                              for j in range(4):
                                  for gi, gname in enumerate("ifgo"):
                                      m = gi * 4 + j
                                      ps = zpp.tile([128, B], dt.float32, tag="z")
                                      if gname == "g":
                                          col = j * 128
                                          if layer == 0:
                                              nc.tensor.matmul(
                                                  ps[:], w1ga_s[:, col:col + 128],
                                                  xhi_s[:, t * B:(t + 1) * B],
                                                  start=True, stop=False)
                                              nc.tensor.matmul(
                                                  ps[:], w1gb_s[:, col:col + 128],
                                                  xlo_s[:, t * B:(t + 1) * B],
                                                  start=False, stop=(t == 0))
                                              if t > 0:
                                                  for k in range(4):
                                                      nc.tensor.matmul(
                                                          ps[:],
                                                          u1g_s[:, k * H + col:
                                                                k * H + col + 128],
                                                          h_prev[k][:],
                                                          start=False, stop=(k == 3))
                                          else:
                                              if t > 0:
                                                  for k in range(4):
                                                      nc.tensor.matmul(
                                                          ps[:],
                                                          u2g_s[:, k * H + col:
                                                                k * H + col + 128],
                                                          h_prev[k][:],
                                                          start=(k == 0), stop=False)
                                              for k in range(4):
                                                  nc.tensor.matmul(
                                                      ps[:],
                                                      w2g_s[:, k * H + col:
                                                            k * H + col + 128],
                                                      h1_new[k][:],
                                                      start=(t == 0 and k == 0),
                                                      stop=(k == 3))
                                      else:
                                          # fp8 DoubleRow path: out [128, 256]
                                          # per n-half, K pairs packed in dim1
                                          c0 = GSLOT[gi] * 512 + j * 128
                                          for nh in range(2):
                                              qd = ps[:, nh * 256:nh * 256 + 256]
                                              n0 = nh * 256
                                              if layer == 0:
                                                  nc.tensor.matmul(
                                                      qd,
                                                      w1p_r[:, :, c0:c0 + 128],
                                                      xp_r[:, :, t * B + n0:
                                                           t * B + n0 + 256],
                                                      start=True,
                                                      stop=(t == 0),
                                                      perf_mode=DR)
                                                  if t > 0:
                                                      for g in range(2):
                                                          nc.tensor.matmul(
                                                              qd,
                                                              u1p_r[g][:, :,
                                                                    c0:c0 + 128],
                                                              hp_prev[g][:]
                                                              .rearrange(
                                                                  "p (i b) -> p i b",
                                                                  i=2)
                                                              [:, :, n0:n0 + 256],
                                                              start=False,
                                                              stop=(g == 1),
                                                              perf_mode=DR)
                                              else:
                                                  if t > 0:
                                                      for g in range(2):
                                                          nc.tensor.matmul(
                                                              qd,
                                                              u2p_r[g][:, :,
                                                                    c0:c0 + 128],
                                                              hp_prev[g][:]
                                                              .rearrange(
                                                                  "p (i b) -> p i b",
                                                                  i=2)
                                                              [:, :, n0:n0 + 256],
                                                              start=(g == 0),
                                                              stop=False,
                                                              perf_mode=DR)
                                                  for g in range(2):
                                                      nc.tensor.matmul(
                                                          qd,
                                                          w2p_r[g][:, :,
                                                                c0:c0 + 128],
                                                          h1p_new[g][:]
                                                          .rearrange(
                                                              "p (i b) -> p i b",
                                                              i=2)
                                                          [:, :, n0:n0 + 256],
                                                          start=(t == 0 and
                                                                 g == 0),
                                                          stop=(g == 1),
                                                          perf_mode=DR)
                                      func = AF.Tanh if gname == "g" else AF.Sigmoid
                                      s = gp.tile([128, B], dt.float32,
                                                  tag=f"s_{gname}")
                                      nc.scalar.activation(
                                          s[:], ps[:], func,
                                          bias=bl_s[:, bcol + m:bcol + m + 1],
                                          scale=(1.0 if gname == "g"
                                                 else 1.0 / WSCALE))
                                      s_g4[gname] = s
                                  # cell update for H-tile j
                                  si, sf, sg, so = (s_g4[g] for g in "ifgo")
                                  if t == 0:
                                      nc.vector.tensor_mul(c_t[j][:], si[:], sg[:])
                                  else:
                                      tig = gp.tile([128, B], dt.float32, tag="tig")
                                      nc.vector.tensor_mul(tig[:], si[:], sg[:])
                                      nc.vector.tensor_mul(c_t[j][:], c_t[j][:], sf[:])
                                      nc.vector.tensor_add(c_t[j][:], c_t[j][:], tig[:])
                                  tc_ = gp.tile([128, B], dt.float32, tag="tc")
                                  nc.scalar.activation(tc_[:], c_t[j][:], AF.Tanh)
                                  nc.vector.tensor_mul(h_new[j][:], so[:], tc_[:])
                                  # fp8 pair-packed copy on the Pool engine
                                  if t < nsteps - 1 or layer == 0:
                                      nc.gpsimd.tensor_copy(
                                          hp_new[j // 2][:, (j % 2) * B:
                                                         (j % 2 + 1) * B],
                                          h_new[j][:])

                      # ---- FC + relu ------------------------------------------
                      for mo in range(2):
                          ps = zpp.tile([128, B], dt.float32, tag="z")
                          for k in range(4):
                              nc.tensor.matmul(
                                  ps[:], fcw_s[k][:, mo * 128:(mo + 1) * 128],
                                  h2_t[nsteps % 2][k][:], start=(k == 0), stop=(k == 3))
                          o_s = gp.tile([128, B], dt.float32, tag="o")
                          nc.scalar.activation(o_s[:], ps[:], AF.Relu,
                                               bias=fcb_s[:, mo:mo + 1])
                          nc.sync.dma_start(out_d[mo * 128:(mo + 1) * 128, :], o_s[:])
                  ls.close()

    nc.finalize()
    return nc


def _pack_dr(Wsel, kparts):
    """[2*kparts, 1536] fp32 -> DoubleRow pack [kparts, 2*1536] fp8."""
    blk = Wsel.reshape(2, kparts, Wsel.shape[1]).transpose(1, 0, 2)
    return np.ascontiguousarray(
        np.clip(blk.reshape(kparts, -1) * WSCALE, -240, 240)).astype(F8)


def _ifo_cols(W):
    """columns for gates i, f, o in pack slot order."""
    return np.concatenate([W[:, 0:H], W[:, H:2 * H], W[:, 3 * H:4 * H]], axis=1)


def prep_consts(conv1_w, conv1_b, conv2_w, conv2_b, W1, U1, b1, W2, U2, b2,
                fc_w, fc_b):
    w1t = np.zeros((K1, 3 * M1), np.float32)
    for dw in range(3):
        for oh in range(OH1):
            for dh in range(7):
                w1t[(oh + dh) * CI:(oh + dh + 1) * CI,
                    dw * M1 + oh * CO1:dw * M1 + (oh + 1) * CO1] = conv1_w[dh, dw]
    # K-stacked conv1 weights: [90, 180]; block A = dw0 (rows 0:45) + dw1
    # (rows 45:90); block B = dw2 in rows 0:45.
    w1t2 = np.zeros((2 * K1, 2 * M1), np.float32)
    w1t2[0:K1, 0:M1] = w1t[:, 0:M1]
    w1t2[K1:2 * K1, 0:M1] = w1t[:, M1:2 * M1]
    w1t2[0:K1, M1:2 * M1] = w1t[:, 2 * M1:3 * M1]

    w2t = np.zeros((K2, 3 * (M2A + M2B)), np.float32)
    for dw in range(3):
        for oh in range(OH2):
            for dh in range(3):
                w2t[(oh + dh) * CO1:(oh + dh + 1) * CO1,
                    dw * 140 + oh * CO2:dw * 140 + (oh + 1) * CO2] = conv2_w[dh, dw]

    W1f, U1f = np.asarray(W1, np.float32), np.asarray(U1, np.float32)
    W2f, U2f = np.asarray(W2, np.float32), np.asarray(U2, np.float32)

    out = {
        "w1t2": w1t2.astype(MM_NP),
        "cb1": np.tile(conv1_b, OH1)[:, None].astype(np.float32),
        "w2t": w2t.astype(MM_NP),
        "cb2a": np.tile(conv2_b, 4)[:, None].astype(np.float32),
        "cb2b": np.tile(conv2_b, 3)[:, None].astype(np.float32),
        "w1p": _pack_dr(_ifo_cols(W1f), 80),
        "w1ga": W1f[0:128, 2 * H:3 * H].astype(MM_NP),
        "w1gb": W1f[128:F, 2 * H:3 * H].astype(MM_NP),
        "bl": np.concatenate([np.asarray(b1).reshape(16, 128).T,
                              np.asarray(b2).reshape(16, 128).T],
                             axis=1).astype(np.float32),
        "fcw": np.asarray(fc_w, np.float32).astype(MM_NP),
        "fcb": np.asarray(fc_b).reshape(2, 128).T.astype(np.float32),
    }
    for nm, Mf in (("u1p", U1f), ("w2p", W2f), ("u2p", U2f)):
        sel = _ifo_cols(Mf)
        for g in range(2):
            out[f"{nm}{g}"] = _pack_dr(sel[256 * g:256 * (g + 1)], 128)
    # bf16 g-gate recurrent weights: [128, 4*H] with k-tile-major columns
    for nm, Mf in (("u1g", U1f), ("w2g", W2f), ("u2g", U2f)):
        gcols = Mf[:, 2 * H:3 * H]           # [512, 512]
        kt = gcols.reshape(4, 128, H).transpose(1, 0, 2).reshape(128, 4 * H)
        out[nm] = np.ascontiguousarray(kt).astype(MM_NP)
    return out


def prep_x(x_shard):
    B = x_shard.shape[0]
    xr = np.asarray(x_shard, np.float32).transpose(1, 3, 0, 2).reshape(K1, B, IW)
    x2 = np.zeros((2 * K1, B, IW), np.float32)
    x2[0:K1] = xr
    x2[K1:2 * K1, :, 0:IW - 1] = xr[:, :, 1:]  # w+1 shifted copy for dw1
    return np.ascontiguousarray(x2.reshape(2 * K1, B * IW)).astype(MM_NP)


_NC_CACHE = {}


def _get_nc(B):
    if B not in _NC_CACHE:
        _NC_CACHE[B] = build_nc(B)
    return _NC_CACHE[B]


def kernel(**inputs):
    from concourse.bass_utils import run_bass_kernel_spmd

    x = np.asarray(inputs["x"])
    Bfull = x.shape[0]
    B = Bfull // N_CORES
    nc = _get_nc(B)
    consts = prep_consts(**{k: np.asarray(v) for k, v in inputs.items()
                            if k != "x"})
    in_maps = []
    for c in range(N_CORES):
        m = dict(consts)
        m["x1"] = prep_x(x[c * B:(c + 1) * B])
        in_maps.append(m)
    res = run_bass_kernel_spmd(nc, in_maps, list(range(N_CORES)))
    out = np.concatenate(
        [res.results[c]["out"].T for c in range(N_CORES)], axis=0)
    return np.ascontiguousarray(out.astype(np.float32))
